# revision 31
# baseline (speedup 1.0000x reference)
"""Grouped SwiGLU expert MLP (MoE) on 8 Trainium2 NeuronCores.

Problem: sorted_x [32768, 512] f32, tokens pre-sorted by expert into 8 equal
contiguous segments of 4096 tokens; per-expert SwiGLU MLP
    h12 = x_e @ w12[e].T          (4096, 2816)
    h   = silu(h12[:, :1408]) * h12[:, 1408:]
    out = h @ w3[e].T             (4096, 512)

Sharding: pure expert parallelism — core e owns expert e's weights and its
4096-token segment (sliced host-side from expert_starts), so no device-side
collectives are needed; the host concatenates the per-core outputs.

Device layout is feature-major throughout ("contraction dim on partitions"),
which makes both GEMMs transpose-free on chip. fp16 operands run the PE at
1 cycle/row (vs 4 for f32); accumulation is always f32 in PSUM.

All DRAM tensors are pre-swizzled on the host into partition-major form
([128, free]) so every DMA lands as 128 long contiguous runs (multi-KB
descriptors) instead of 512 short ones — this matters for the startup
latency, where the PE is waiting on the first weight/activation chunks.
w12 columns are pre-interleaved in (gate 128 | up 128) pairs so one DMA
chunk feeds both halves of each SwiGLU band.

Measured device model this schedule is built around (from perfetto):
  - DMA descriptors are served in ONE global FIFO by issue order, at
    ~110ns + ~22.5ns/KB each across 16 queues (~200GB/s for 2KB runs,
    ~440GB/s for 8KB runs).  Whatever is issued first lands first, so
    the critical x(0) + w12 pair 0 must be the only early descriptors.
  - The Tile scheduler is a list scheduler: a dep-blocked DMA does not
    hold back later dep-free DMAs on the same queue, so every bulk
    transfer carries an explicit dependency (gate) on x(0).
  - The PE clock ramps for ~3.5us (matmuls run ~2x slow) after going
    busy, and drops back after idle gaps — so the head runs warm-up
    matmuls on zeroed scratch sized to end exactly when x(0) lands.
  - GEMM2 is software-pipelined into the GEMM1/SwiGLU loop with LAG=4
    (relaxes the w3 arrival deadline at startup).
  - The last token block runs GEMM2 in do-major order, with the final
    column pair split into two half-token chains in separate PSUM
    tiles, so nearly all of the output drains while the PE still works;
    the post-matmul tail is one [128,256] copy + one 128KB store.
"""

import os

import numpy as np

import concourse.bass as bass
import concourse.mybir as mybir
import concourse.tile as tile
from concourse import bacc
from concourse.bass_utils import run_bass_kernel_spmd

N_CORES = 8
D = 512  # d_model
H = 1408  # hidden
TWOH = 2 * H
TPE = 4096  # tokens per expert
NT = 512  # token block (matmul moving free dim, one PSUM bank in f32)
KD = D // 128  # 4 contraction tiles over d
KH = H // 128  # 11 contraction tiles over h
NB = TPE // NT  # token blocks
PAIRW = 256  # interleaved gate|up column pair width
XBLK = KD * NT  # 2048 fp16 elems per x token-block per partition

F16 = mybir.dt.float16
F32 = mybir.dt.float32
NP_F16 = np.dtype(np.float16)

# Results of a traced run (test harness reads these).
last_exec_time_ns = None
last_trace_path = None


def _build():
    # Bacc (not plain Bass): its compile() pass pipeline legalizes sync
    # waits (>=2 waits per instruction are split into event-sem chains),
    # which this image's walrus requires.
    nc = bacc.Bacc("TRN2", target_bir_lowering=False, debug=False, num_devices=N_CORES)
    xh = nc.dram_tensor("xh", [128, NB * XBLK], F16, kind="ExternalInput")
    w12h = nc.dram_tensor("w12h", [128, KH * KD * PAIRW], F16, kind="ExternalInput")
    w3h = nc.dram_tensor("w3h", [128, KH * D], F16, kind="ExternalInput")
    oh = nc.dram_tensor("oh", [128, NB * KD * NT], F32, kind="ExternalOutput")

    LAG = 4

    with tile.TileContext(nc) as tc:
        with (
            tc.tile_pool(name="weights", bufs=1) as wpool,
            tc.tile_pool(name="xin", bufs=1) as xpool,
            tc.tile_pool(name="ht", bufs=2) as hpool,
            tc.tile_pool(name="swi", bufs=4) as spool,
            tc.tile_pool(name="ot", bufs=4) as opool,
            tc.tile_pool(name="pg", bufs=2, space=bass.MemorySpace.PSUM) as pgate,
            tc.tile_pool(name="pu", bufs=2, space=bass.MemorySpace.PSUM) as pup,
            tc.tile_pool(name="po", bufs=1, space=bass.MemorySpace.PSUM) as pacc,
        ):
            w12s = wpool.tile([128, KH, KD * PAIRW], F16)
            w3s = wpool.tile([128, KH, D], F16)
            xs = xpool.tile([128, NB, XBLK], F16)

            def dma_xr(q, a, b):
                q.dma_start(out=xs[:, a:b, :], in_=xh[:, a * XBLK : b * XBLK])

            def dma_w12(q, a, b):
                q.dma_start(
                    out=w12s[:, a:b, :],
                    in_=w12h[:, a * KD * PAIRW : b * KD * PAIRW],
                )

            def dma_w3(q, a, b):
                q.dma_start(out=w3s[:, a:b, :], in_=w3h[:, a * D : b * D])

            # Prologue.  DMA facts (measured): descriptors are served in
            # strict FIFO by issue order across the whole device at
            # ~110ns + ~22.5ns/KB each spread over 16 queues, i.e. there
            # is ONE effective service order — whatever is issued first
            # transfers first, at ~200GB/s for 2KB runs up to ~440GB/s
            # for 8KB runs.  So the sync queue issues exactly the
            # critical-path sequence: w12 pair 0, then x(0) in two halves
            # (the PE can start GEMM1 hh=0 on kd 0-1 as soon as the first
            # half + pair 0 are in, ~9.7us), then pairs 1-2 which GEMM1
            # consumes at ramp speed.  Everything else sits on the Pool
            # queue behind a one-instruction gate: a 2-element DVE copy
            # that reads x(0)'s tile (RAW on both x(0) half-DMAs) and
            # writes into the first bulk chunk's destination (WAW), so no
            # bulk descriptor enters the service FIFO before x(0) lands.
            # PE warm-up: ~4us of matmuls on zeroed scratch (no DMA deps)
            # so the tensor engine's p-state ramp (~3us at half speed)
            # completes while the critical DMAs are still in flight, and
            # every real matmul runs at full clock.  Sized to end just
            # before x(0) lands (~12us).
            scr = wpool.tile([128, 640], F16)
            nc.gpsimd.memset(scr[:], 0.0)
            wps = pacc.tile([128, NT], F32, name="warm", tag="acc0")
            for _ in range(9):
                nc.tensor.matmul(
                    wps[:], scr[:, 0:128], scr[:, 128:640], start=True, stop=True
                )
            wsink = wpool.tile([128, 2], F32)
            nc.scalar.copy(wsink[:], wps[:, 0:2])  # reader so wps releases

            # pair 0 on the scalar HWDGE queue, x(0) on sync: their issues
            # complete ~7.5/7.9us so pair 0's short service slots in just
            # ahead of x(0)'s in the descriptor FIFO — both land ~11us
            # instead of serializing behind a single queue's issues.
            dma_w12(nc.scalar, 0, 1)  # pair 0 (hh=0): 2KB runs
            nc.sync.dma_start(  # x(0): one transfer, one completion sem
                out=xs[:, 0, :], in_=xh[:, 0:XBLK]
            )
            dma_w12(nc.scalar, 1, 3)  # pairs 1-2
            # pairs 3-4 + w3 kh0-2 also ungated: their descriptors queue
            # BEHIND x(0) in the FIFO (so they can't delay it) but don't
            # depend on gate timing — removes the occasional 0.5us GEMM1
            # stall when the gated bulk stream starts late
            dma_w12(nc.scalar, 3, 5)
            dma_w3(nc.scalar, 0, 3)
            # Gates: the Tile scheduler is a list scheduler — a blocked
            # instruction does NOT hold back later dep-free instructions on
            # the same queue — so EVERY bulk DMA gets its own dependency on
            # x(0): a 2-element copy that reads across x(0)'s half-DMA
            # boundary (RAW) and scribbles into that DMA's destination
            # (WAW, overwritten by the DMA itself).  This keeps every bulk
            # descriptor out of the service FIFO until x(0) has landed.
            # Weight-chunk gates ride the idle DVE, x-chunk gates the idle
            # ACT, so neither delays the first SwiGLU.
            xgate = xs[:, 0, XBLK // 2 - 1 : XBLK // 2 + 1]
            BULK_W = [(5, 7), (7, 9), (3, 7), (9, 11), (7, 11)]
            #          w12     w12     w3      w12      w3
            for i, (a, b) in enumerate(BULK_W):
                is12 = i in (0, 1, 3)
                dst = w12s if is12 else w3s
                nc.vector.tensor_copy(dst[:, a, 0:2], xgate)
                if is12:
                    dma_w12(nc.gpsimd, a, b)
                else:
                    dma_w3(nc.gpsimd, a, b)
            for a, b in [(1, 3), (3, 5), (5, 7), (7, 8)]:
                nc.scalar.copy(xs[:, a, 0:2], xgate)
                dma_xr(nc.gpsimd, a, b)

            for tb in range(NB):
                ht = hpool.tile([128, KH, NT], F16)
                acc = [
                    pacc.tile([128, NT], F32, name=f"acc{do}", tag=f"acc{do}")
                    for do in range(KD)
                ]

                def mm_g(ps_g, hh, kd):
                    nc.tensor.matmul(
                        ps_g[:],
                        w12s[:, hh, kd * PAIRW : kd * PAIRW + 128],
                        xs[:, tb, kd * NT : (kd + 1) * NT],
                        start=(kd == 0),
                        stop=(kd == KD - 1),
                    )

                def mm_u(ps_u, hh, kd):
                    nc.tensor.matmul(
                        ps_u[:],
                        w12s[:, hh, kd * PAIRW + 128 : (kd + 1) * PAIRW],
                        xs[:, tb, kd * NT : (kd + 1) * NT],
                        start=(kd == 0),
                        stop=(kd == KD - 1),
                    )

                def gemm1_swiglu(hh):
                    ps_g = pgate.tile([128, NT], F32)
                    ps_u = pup.tile([128, NT], F32)
                    for kd in range(KD):
                        mm_g(ps_g, hh, kd)
                    for kd in range(KD):
                        mm_u(ps_u, hh, kd)
                    sil = spool.tile([128, NT], F32)
                    nc.scalar.activation(
                        sil[:], ps_g[:], mybir.ActivationFunctionType.Silu
                    )
                    nc.vector.tensor_mul(ht[:, hh, :], sil[:], ps_u[:])

                if tb < NB - 1:
                    # kh-major GEMM2 pipelined into the GEMM1 loop: in
                    # iteration hh we consume ht[hh - LAG], so the PE never
                    # waits on the ACT+DVE SwiGLU chain.
                    def gemm2_step(kh):
                        for do in range(KD):
                            nc.tensor.matmul(
                                acc[do][:],
                                w3s[:, kh, do * 128 : (do + 1) * 128],
                                ht[:, kh, :],
                                start=(kh == 0),
                                stop=(kh == KH - 1),
                            )

                    for hh in range(KH):
                        gemm1_swiglu(hh)
                        if hh >= LAG:
                            gemm2_step(hh - LAG)
                    for kh in range(KH - LAG, KH):
                        gemm2_step(kh)

                    # PSUM->SBUF copies split across ACT and DVE; one
                    # coalesced output DMA per block.
                    ot = opool.tile([128, KD * NT], F32)
                    for do in range(KD):
                        if do % 2 == 0:
                            nc.scalar.copy(ot[:, do * NT : (do + 1) * NT], acc[do][:])
                        else:
                            nc.vector.tensor_copy(
                                ot[:, do * NT : (do + 1) * NT], acc[do][:]
                            )
                    nc.sync.dma_start(
                        out=oh[:, tb * XBLK : (tb + 1) * XBLK], in_=ot[:]
                    )
                else:
                    # Last block: GEMM1 first, then GEMM2 in do-major order
                    # so acc[0..2] finish (and drain via copy+DMA) while the
                    # PE is still on acc[1..3].  The post-PE tail is then a
                    # single [128,512] copy + one 256KB DMA.
                    for hh in range(KH):
                        gemm1_swiglu(hh)
                    ot = opool.tile([128, KD * NT], F32)
                    for do in range(3):
                        for kh in range(KH):
                            nc.tensor.matmul(
                                acc[do][:],
                                w3s[:, kh, do * 128 : (do + 1) * 128],
                                ht[:, kh, :],
                                start=(kh == 0),
                                stop=(kh == KH - 1),
                            )
                        if do % 2 == 0:
                            nc.scalar.copy(ot[:, do * NT : (do + 1) * NT], acc[do][:])
                            nc.scalar.dma_start(
                                out=oh[:, tb * XBLK + do * NT : tb * XBLK + (do + 1) * NT],
                                in_=ot[:, do * NT : (do + 1) * NT],
                            )
                        else:
                            nc.vector.tensor_copy(
                                ot[:, do * NT : (do + 1) * NT], acc[do][:]
                            )
                            nc.sync.dma_start(
                                out=oh[:, tb * XBLK + do * NT : tb * XBLK + (do + 1) * NT],
                                in_=ot[:, do * NT : (do + 1) * NT],
                            )
                    # do=3 runs as two half-token accumulation chains in
                    # SEPARATE PSUM tiles (PSUM deps are tile-granular, so
                    # chain A's drain must not share a tile with chain B).
                    # Chain B reuses tag acc0's bank — free since do=0
                    # drained long ago.  The post-PE tail is one [128,256]
                    # copy + one 128KB store.
                    NH = NT // 2
                    o3 = tb * XBLK + 3 * NT
                    h3b = pacc.tile([128, NT], F32, name="acc3b", tag="acc0")
                    for half, (c0, c1) in enumerate([(0, NH), (NH, NT)]):
                        dst = acc[3] if half == 0 else h3b
                        for kh in range(KH):
                            nc.tensor.matmul(
                                dst[:, 0:NH],
                                w3s[:, kh, 3 * 128 : 4 * 128],
                                ht[:, kh, c0:c1],
                                start=(kh == 0),
                                stop=(kh == KH - 1),
                            )
                        if half == 0:
                            nc.scalar.copy(
                                ot[:, 3 * NT + c0 : 3 * NT + c1], dst[:, 0:NH]
                            )
                            nc.scalar.dma_start(
                                out=oh[:, o3 + c0 : o3 + c1],
                                in_=ot[:, 3 * NT + c0 : 3 * NT + c1],
                            )

                        else:
                            nc.vector.tensor_copy(
                                ot[:, 3 * NT + c0 : 3 * NT + c1], dst[:, 0:NH]
                            )
                            # scalar queue: its ring was active ~1us ago
                            # (chain A's store) so the doorbell is warm,
                            # unlike sync which has been idle for ~8us
                            nc.scalar.dma_start(
                                out=oh[:, o3 + c0 : o3 + c1],
                                in_=ot[:, 3 * NT + c0 : 3 * NT + c1],
                            )
    nc.compile()
    return nc


_nc_cache = None


def _get_nc():
    global _nc_cache
    if _nc_cache is None:
        _nc_cache = _build()
    return _nc_cache


# Column permutation interleaving gate/up weight columns in 128-wide pairs:
# perm[256j + k] = 128j + k (k < 128, gate) | 1408 + 128j + k - 128 (up).
_PERM = np.empty(TWOH, dtype=np.int64)
for _j in range(KH):
    _PERM[PAIRW * _j : PAIRW * _j + 128] = np.arange(128) + 128 * _j
    _PERM[PAIRW * _j + 128 : PAIRW * (_j + 1)] = np.arange(128) + H + 128 * _j


def kernel(sorted_x, w12, w3, expert_starts, expert_ends):
    global last_exec_time_ns, last_trace_path
    sorted_x = np.asarray(sorted_x)
    w12 = np.asarray(w12)
    w3 = np.asarray(w3)
    starts = np.asarray(expert_starts).astype(np.int64)
    T = sorted_x.shape[0]

    in_maps = []
    for e in range(N_CORES):
        # jax.lax.dynamic_slice clamps the start index the same way
        s = int(min(max(starts[e], 0), T - TPE))
        xe = sorted_x[s : s + TPE]  # (TPE, D) f32

        # xh[p, tb*XBLK + kd*NT + t] = xe[tb*NT + t, kd*128 + p]
        xt = np.ascontiguousarray(xe.T).astype(NP_F16)  # [512, 4096]
        xhost = np.ascontiguousarray(
            xt.reshape(KD, 128, NB, NT).transpose(1, 2, 0, 3).reshape(128, NB * XBLK)
        )

        # w12h[p, j*KD*PAIRW + kd*PAIRW + c] = w12[e][perm[256j + c], kd*128 + p]
        W = w12[e].T[:, _PERM].astype(NP_F16)  # [512, 2816] interleaved cols
        w12host = np.ascontiguousarray(
            W.reshape(KD, 128, KH, PAIRW)
            .transpose(1, 2, 0, 3)
            .reshape(128, KH * KD * PAIRW)
        )

        # w3h[p, kh*D + d] = w3[e][d, kh*128 + p]
        w3t = w3[e].T.astype(NP_F16)  # [1408, 512]
        w3host = np.ascontiguousarray(
            w3t.reshape(KH, 128, D).transpose(1, 0, 2).reshape(128, KH * D)
        )

        in_maps.append({"xh": xhost, "w12h": w12host, "w3h": w3host})

    trace = bool(os.environ.get("BASS_MOE_TRACE"))
    res = run_bass_kernel_spmd(
        _get_nc(), in_maps, core_ids=list(range(N_CORES)), trace=trace
    )
    if trace:
        last_exec_time_ns = res.exec_time_ns
        iat = res.instructions_and_trace
        last_trace_path = iat[1] if iat else None

    out = np.empty((N_CORES * TPE, D), dtype=np.float32)
    for e in range(N_CORES):
        # oh[p, tb*XBLK + do*NT + t] = out_e[tb*NT + t, do*128 + p]
        ohe = res.results[e]["oh"].reshape(128, NB, KD, NT)
        out[e * TPE : (e + 1) * TPE] = (
            ohe.transpose(1, 3, 2, 0).reshape(TPE, D)
        )
    return out
